# revision 8
# baseline (speedup 1.0000x reference)
"""Diagonal-masked multi-head self-attention on 8 TRN2 NeuronCores.

Sharding: core c handles batch b = c // 2 and heads h0 = (c % 2) * 8 .. +8
(data parallel on B=4, tensor parallel over the 16 heads).  Each core
computes a partial output [S, D]; the host sums the two half-head partials
per batch and adds the output bias.

v2 schedule: software-pipelined across phases so the PE never drains.
  head:   K(0) proj, V proj (all pairs), Q(0) proj
  stage p: attention for pair p; K/Q projections for pair p+1 are emitted
           as PE filler between q-chunks; in the last stage the output
           projection chunks are interleaved per completed q-chunk.
Inside attention the score matmul for t+1 is emitted before the PV matmul
for t, so the PE streams scores while the scalar engine runs exp(t).
The diagonal mask is one fused [128,1024] bf16 multiply with a
precomputed mask tile.  All PSUM->SBUF copies run on DVE; the scalar
engine does nothing but exp.

v3: the two heads of a pair are row-tiled onto the top/bottom 64-row
halves of the PE array (DK=64 contraction), so a score-tile pair costs
one matmul slot instead of two zero-padded ones.
"""

import numpy as np
import ml_dtypes

B, S, D, H = 4, 2048, 1024, 16
DK = D // H
N_CORES = 8
HEADS_PER_CORE = H // 2


def build_attention_core(S=2048, DIN=1024, NH=8, DOUT=1024, aug_bias=False):
    import concourse.bacc as bacc
    import concourse.bass as bass
    import concourse.mybir as mybir
    import concourse.tile as tile

    fp32 = mybir.dt.float32
    bf16 = mybir.dt.bfloat16

    NP = NH // 2              # head pairs
    DC = NH * DK              # concat head dim on this core
    VW = 128                  # per-head V slot: [V(64) ones(1) pad(63)]
    NT = S // 128             # t tiles (key/value positions)
    NQ = S // 512             # q chunks of 512
    KA = DIN + 1 if aug_bias else DIN
    NK = (KA + 127) // 128    # contraction tiles for projections
    QT = S // 128             # output q tiles

    assert S % 512 == 0 and DIN % 128 == 0 and DOUT == 1024

    nc = bacc.Bacc(None, target_bir_lowering=False, debug=False)

    xq = nc.dram_tensor("xq", [KA, S], bf16, kind="ExternalInput")
    xk = nc.dram_tensor("xk", [KA, S], bf16, kind="ExternalInput")
    xv = nc.dram_tensor("xv", [KA, S], bf16, kind="ExternalInput")
    wq = nc.dram_tensor("wq", [KA, DC], bf16, kind="ExternalInput")
    wk = nc.dram_tensor("wk", [KA, DC], bf16, kind="ExternalInput")
    wv = nc.dram_tensor("wv", [KA, DC], bf16, kind="ExternalInput")
    wo = nc.dram_tensor("wo", [DC, DOUT], bf16, kind="ExternalInput")
    dmk = nc.dram_tensor("dmk", [128, 4 * 1024], bf16, kind="ExternalInput")
    outp = nc.dram_tensor("outp", [S, DOUT], fp32, kind="ExternalOutput")

    def ksz(k):  # rows in contraction tile k
        return min(128, KA - k * 128)

    scale = float(1.0 / np.sqrt(DK))

    with tile.TileContext(nc) as tc:
        with (
            tc.tile_pool(name="persist", bufs=1) as persist,
            tc.tile_pool(name="xin", bufs=32) as xin,
            tc.tile_pool(name="win", bufs=1) as win,
            tc.tile_pool(name="epool", bufs=4) as epool,
            tc.tile_pool(name="npool", bufs=2) as npool,
            tc.tile_pool(name="opool", bufs=2) as opool,
            tc.tile_pool(name="scps", bufs=2, space="PSUM") as scps,
            tc.tile_pool(name="otaps", bufs=2, space="PSUM") as otaps,
            tc.tile_pool(name="otbps", bufs=2, space="PSUM") as otbps,
        ):
            # ---- persistent SBUF tensors -------------------------------
            qht = persist.tile([128, NP * S], bf16, tag="qht")        # pair-major
            kht = persist.tile([128, NP * S], bf16, tag="kht")        # pair-major
            vh = persist.tile([128, NH * NT * VW], bf16, tag="vh")    # head-major
            ot = persist.tile([128, NP * S], bf16, tag="ot")
            dmask = persist.tile([128, 4 * 1024], bf16, tag="dmask")
            wo_sb = persist.tile([128, NP * DOUT], bf16, tag="wo")

            # K-path DMAs first so the first projection chain starts ASAP;
            # everything else is emitted behind them.
            wt = {}

            def dma_w(which, wdram):
                for k in range(NK):
                    wtile = win.tile([128, DC], bf16, tag=f"w{which}{k}")
                    nc.sync.dma_start(
                        wtile[: ksz(k), :], wdram[k * 128: k * 128 + ksz(k), :]
                    )
                    wt[(which, k)] = wtile

            vh4 = vh.rearrange("p (h t c) -> p h t c", t=NT, c=VW)

            # ---------------- helper emitters ---------------------------
            def dma_x(xdram, n, tagless_list):
                """DMA k-tiles of column-chunk n of xdram into pool tiles."""
                tiles = []
                for k in range(NK):
                    t_ = xin.tile([128, 512], bf16, tag="x")
                    nc.sync.dma_start(
                        t_[: ksz(k), :],
                        xdram[k * 128: k * 128 + ksz(k), n * 512:(n + 1) * 512],
                    )
                    tiles.append(t_)
                tagless_list.append(tiles)
                return tiles

            def proj_kq_fillers(which, p, n, xtiles):
                """Closures (one matmul each) for a K/Q projection chain."""
                pool = otbps if which == "q" else otaps
                tag = "otb" if which == "q" else "ota"
                box = {}

                def mk(k):
                    def emit():
                        if k == 0:
                            box["ps"] = pool.tile([128, 512], fp32, tag=tag, name="pjps")
                        ps = box["ps"]
                        nc.tensor.matmul(
                            ps[:],
                            wt[(which, k)][: ksz(k), p * 128:(p + 1) * 128],
                            xtiles[k][: ksz(k), :],
                            start=(k == 0),
                            stop=(k == NK - 1),
                        )
                        if k == NK - 1:
                            dst = qht if which == "q" else kht
                            nc.vector.tensor_copy(
                                dst[:, p * S + n * 512: p * S + (n + 1) * 512],
                                ps[:],
                            )

                    return emit

                return [mk(k) for k in range(NK)]

            def proj_kq(which, p, n, xtiles):
                for f in proj_kq_fillers(which, p, n, xtiles):
                    f()

            def proj_v(n, xtiles):
                """V projection for the 4 t-tiles of chunk n (all heads)."""
                for tt in range(4):
                    t = n * 4 + tt
                    ps = scps.tile([128, 1024], fp32, tag="sc")
                    for k in range(NK):
                        nc.tensor.matmul(
                            ps[:, 0:512],
                            xtiles[k][: ksz(k), tt * 128:(tt + 1) * 128],
                            wt[("v", k)][: ksz(k), :],
                            start=(k == 0),
                            stop=(k == NK - 1),
                        )
                    nc.vector.tensor_copy(
                        vh4[:, :, t, 0:DK],
                        ps[:, 0:512].rearrange("p (h c) -> p h c", c=DK),
                    )

            def sc_mm(p, n):
                """Emit the score matmul pair for (pair p, chunk n, tile t).

                DK=64, so each head's score matmul only needs 64 contraction
                rows; the two heads of a pair are row-tiled onto halves of
                the PE array (tile_position (0,0) / (64,0), inferred from the
                base partitions) and run concurrently."""
                qof = p * S + n * 512
                kof = p * S

                def emit(t):
                    sc = scps.tile([128, 1024], fp32, tag="sc")
                    nc.tensor.matmul(
                        sc[:, 0:512],
                        kht[0:64, kof + t * 128: kof + (t + 1) * 128],
                        qht[0:64, qof: qof + 512],
                        start=True, stop=True,
                    )
                    nc.tensor.matmul(
                        sc[:, 512:1024],
                        kht[64:128, kof + t * 128: kof + (t + 1) * 128],
                        qht[64:128, qof: qof + 512],
                        start=True, stop=True,
                    )
                    return sc

                return emit

            pend = {}

            def attn_chunk(p, n, fillers=(), nxt=None):
                """Attention for pair p, q-chunk n (512 q positions).

                fillers: closures, each emitting ~1 PE matmul, interleaved one
                per t-iteration to fill the exp-wait slack.
                nxt: (p', n') of the following chunk; its first score matmul
                pair is emitted just before this chunk's last PV so the
                scalar engine never idles across the boundary."""
                qof = p * S + n * 512
                ota = otaps.tile([128, 512], fp32, tag="ota")
                otb = otbps.tile([128, 512], fp32, tag="otb")
                mine = sc_mm(p, n)
                sc_cur = pend.pop("sc", None)
                if sc_cur is None:
                    sc_cur = mine(0)
                fq = list(fillers)
                nf = len(fq)
                emitted = 0
                for t in range(NT):
                    e = epool.tile([128, 1024], bf16, tag="e")
                    nc.scalar.activation(
                        e[:], sc_cur[:], mybir.ActivationFunctionType.Exp,
                        scale=scale,
                    )
                    off = t * 128 - n * 512
                    if 0 <= off < 512:
                        d = off // 128
                        nc.vector.tensor_mul(
                            e[:], e[:], dmask[:, d * 1024:(d + 1) * 1024]
                        )
                    if t < NT - 1:
                        sc_cur = mine(t + 1)
                    elif nxt is not None:
                        pend["sc"] = sc_mm(*nxt)(0)
                    while emitted < ((t + 1) * nf) // NT:
                        fq[emitted]()
                        emitted += 1
                    va = ((2 * p) * NT + t) * VW
                    vb = ((2 * p + 1) * NT + t) * VW
                    nc.tensor.matmul(
                        ota[:], vh[:, va: va + VW], e[:, 0:512],
                        start=(t == 0), stop=(t == NT - 1),
                    )
                    nc.tensor.matmul(
                        otb[:], vh[:, vb: vb + VW], e[:, 512:1024],
                        start=(t == 0), stop=(t == NT - 1),
                    )
                while emitted < nf:
                    fq[emitted]()
                    emitted += 1

                # normalize (denominators on PSUM row 64)
                rd = npool.tile([128, 1024], fp32, tag="rd")
                nc.vector.reciprocal_approx_fast(rd[:, 0:512], ota[:])
                nc.vector.reciprocal_approx_fast(rd[:, 512:1024], otb[:])
                nc.sync.dma_start(rd[0:1, 0:512], rd[64:65, 0:512])
                nc.sync.dma_start(rd[0:1, 512:1024], rd[64:65, 512:1024])
                bca = npool.tile([64, 512], fp32, tag="bca")
                bcb = npool.tile([64, 512], fp32, tag="bcb")
                nc.gpsimd.partition_broadcast(bca[:], rd[0:1, 0:512], channels=64)
                nc.gpsimd.partition_broadcast(bcb[:], rd[0:1, 512:1024], channels=64)
                nc.vector.tensor_mul(ot[0:64, qof: qof + 512], ota[0:64, :], bca[:])
                tmpb = npool.tile([64, 512], bf16, tag="tmpb")
                nc.vector.tensor_mul(tmpb[:], otb[0:64, :], bcb[:])
                nc.sync.dma_start(ot[64:128, qof: qof + 512], tmpb[:])

            def out_qt_closure(qt):
                """One output-projection q-subtile (8 matmuls + copy + DMA)."""

                def emit():
                    ps = scps.tile([128, 1024], fp32, tag="sc", name="cps")
                    for nd in range(2):
                        for p in range(NP):
                            nc.tensor.matmul(
                                ps[:, nd * 512:(nd + 1) * 512],
                                ot[:, p * S + qt * 128: p * S + (qt + 1) * 128],
                                wo_sb[:, p * DOUT + nd * 512: p * DOUT + nd * 512 + 512],
                                start=(p == 0), stop=(p == NP - 1),
                            )
                    osb = opool.tile([128, 1024], fp32, tag="osb", name="osb")
                    nc.vector.tensor_copy(osb[:], ps[:])
                    nc.sync.dma_start(outp[qt * 128:(qt + 1) * 128, :], osb[:])

                return emit

            def out_chunk(n):
                for qt in range(n * 4, n * 4 + 4):
                    out_qt_closure(qt)()

            # ---------------- emission ----------------------------------
            keep = []
            # head: K(0), V(all), Q(0); DMAs in dependency-first order
            dma_w("k", wk)
            kx = [dma_x(xk, n, keep) for n in range(NQ)]
            dma_w("v", wv)
            nc.vector.memset(vh4[:, :, :, 64:65], 1.0)  # ones columns only
            proj_kq("k", 0, 0, kx[0])
            vx = [dma_x(xv, n, keep) for n in range(NQ)]
            for n in range(1, NQ):
                proj_kq("k", 0, n, kx[n])
            # remaining startup work, off the critical DMA path
            nc.sync.dma_start(dmask[:], dmk[:])
            dma_w("q", wq)
            for p in range(NP):
                nc.sync.dma_start(
                    wo_sb[:, p * DOUT:(p + 1) * DOUT], wo[p * 128:(p + 1) * 128, :]
                )
            for n in range(NQ):
                proj_v(n, vx[n])
            qx = [dma_x(xq, n, keep) for n in range(NQ)]
            # rolling x prefetch for the first stage's fillers
            xbuf = {0: (dma_x(xk, 0, keep), dma_x(xq, 0, keep))}
            for n in range(NQ):
                proj_kq("q", 0, n, qx[n])

            # stages: chunk (p, n) with next-chunk score lookahead,
            # next-pair projections (or output-projection subtiles in the
            # last stage) interleaved into the t-loop, and x chunks
            # DMA'd one chunk ahead
            order = [(p, n) for p in range(NP) for n in range(NQ)]
            for idx, (p, n) in enumerate(order):
                if idx + 1 < len(order) and order[idx + 1][0] < NP - 1:
                    n2 = order[idx + 1][1]
                    xbuf[idx + 1] = (dma_x(xk, n2, keep), dma_x(xq, n2, keep))
                fillers = []
                if p < NP - 1:
                    kxn, qxn = xbuf.pop(idx)
                    fillers = (
                        proj_kq_fillers("k", p + 1, n, kxn)
                        + proj_kq_fillers("q", p + 1, n, qxn)
                    )
                elif n > 0:
                    fillers = [out_qt_closure(qt)
                               for qt in range((n - 1) * 4, n * 4)]
                nxt = order[idx + 1] if idx + 1 < len(order) else None
                attn_chunk(p, n, fillers, nxt)
            out_chunk(NQ - 1)

    nc.compile()
    return nc


def _bf16(a):
    return np.ascontiguousarray(a).astype(ml_dtypes.bfloat16)


def _build_dmask():
    m = np.ones((128, 4 * 1024), np.float32)
    for d in range(4):
        for i in range(128):
            m[i, d * 1024 + d * 128 + i] = 0.0
            m[i, d * 1024 + 512 + d * 128 + i] = 0.0
    return _bf16(m)


def _prep_core_inputs(q, k, v, Wq, bq, Wk, bk, Wv, bv, Wo, aug_bias):
    """Per-core host-side slicing/transposition. Returns list of 8 dicts."""
    dmk = _build_dmask()
    maps = []
    for c in range(N_CORES):
        b = c // 2
        h0 = (c % 2) * HEADS_PER_CORE
        r0, r1 = h0 * DK, (h0 + HEADS_PER_CORE) * DK
        m = {}
        for name, x in (("xq", q[b]), ("xk", k[b]), ("xv", v[b])):
            xt = x.T  # [D, S]
            if aug_bias:
                xt = np.concatenate([xt, np.ones((1, S), np.float32)], axis=0)
            m[name] = _bf16(xt)
        for name, W, bias in (("wq", Wq, bq), ("wk", Wk, bk), ("wv", Wv, bv)):
            wtm = W[r0:r1, :].T  # [D, DC]
            if aug_bias:
                wtm = np.concatenate([wtm, bias[None, r0:r1]], axis=0)
            m[name] = _bf16(wtm)
        m["wo"] = _bf16(Wo[:, r0:r1].T)  # [DC, D]
        m["dmk"] = dmk
        maps.append(m)
    return maps


_PROGRAM_CACHE = {}


def _get_program(aug_bias):
    if aug_bias not in _PROGRAM_CACHE:
        _PROGRAM_CACHE[aug_bias] = build_attention_core(
            S=S, DIN=D, NH=HEADS_PER_CORE, DOUT=D, aug_bias=aug_bias
        )
    return _PROGRAM_CACHE[aug_bias]


def _reference_fallback(q, k, v, Wq, bq, Wk, bk, Wv, bv, Wo, bo, mask):
    """Pure-numpy fallback for unexpected mask patterns."""
    out = np.empty((B, S, D), np.float32)
    msk = np.broadcast_to(mask.reshape(mask.shape[-2], mask.shape[-1]), (S, S))
    for b in range(B):
        qh = (q[b] @ Wq.T + bq).reshape(S, H, DK).transpose(1, 0, 2)
        kh = (k[b] @ Wk.T + bk).reshape(S, H, DK).transpose(1, 0, 2)
        vh = (v[b] @ Wv.T + bv).reshape(S, H, DK).transpose(1, 0, 2)
        acc = np.empty((H, S, DK), np.float32)
        for h in range(H):
            s = (qh[h] @ kh[h].T) / np.float32(np.sqrt(DK))
            s = np.where(msk == 0, np.finfo(np.float32).min, s)
            s = s - s.max(axis=-1, keepdims=True)
            e = np.exp(s)
            p = e / e.sum(axis=-1, keepdims=True)
            acc[h] = p @ vh[h]
        o = acc.transpose(1, 0, 2).reshape(S, D)
        out[b] = o @ Wo.T + bo
    return out


def kernel(q, k, v, Wq, bq, Wk, bk, Wv, bv, Wo, bo, mask, _trace=False):
    from concourse.bass_utils import run_bass_kernel_spmd

    q = np.asarray(q, np.float32)
    k = np.asarray(k, np.float32)
    v = np.asarray(v, np.float32)
    Wq, bq = np.asarray(Wq, np.float32), np.asarray(bq, np.float32)
    Wk, bk = np.asarray(Wk, np.float32), np.asarray(bk, np.float32)
    Wv, bv = np.asarray(Wv, np.float32), np.asarray(bv, np.float32)
    Wo, bo = np.asarray(Wo, np.float32), np.asarray(bo, np.float32)
    mask = np.asarray(mask)

    expected_mask = 1 - np.eye(S, dtype=np.int32)
    if not np.array_equal(mask.reshape(-1, S, S)[0].astype(np.int32), expected_mask):
        return _reference_fallback(q, k, v, Wq, bq, Wk, bk, Wv, bv, Wo, bo, mask)

    aug_bias = bool(np.any(bq) or np.any(bk) or np.any(bv))
    nc = _get_program(aug_bias)
    in_maps = _prep_core_inputs(q, k, v, Wq, bq, Wk, bk, Wv, bv, Wo, aug_bias)
    res = run_bass_kernel_spmd(
        nc, in_maps, core_ids=list(range(N_CORES)), trace=_trace
    )
    out = np.empty((B, S, D), np.float32)
    for b in range(B):
        out[b] = res.results[2 * b]["outp"] + res.results[2 * b + 1]["outp"] + bo
    if _trace:
        kernel.last_results = res
    return out



# revision 11
# speedup vs baseline: 1.0050x; 1.0050x over previous
"""Diagonal-masked multi-head self-attention on 8 TRN2 NeuronCores.

Sharding: core c handles batch b = c // 2 and heads h0 = (c % 2) * 8 .. +8
(data parallel on B=4, tensor parallel over the 16 heads).  Each core
computes a partial output [S, D]; the host sums the two half-head partials
per batch and adds the output bias.

v2 schedule: software-pipelined across phases so the PE never drains.
  head:   K(0) proj, V proj (all pairs), Q(0) proj
  stage p: attention for pair p; K/Q projections for pair p+1 are emitted
           as PE filler between q-chunks; in the last stage the output
           projection chunks are interleaved per completed q-chunk.
Inside attention the score matmul for t+1 is emitted before the PV matmul
for t, so the PE streams scores while the scalar engine runs exp(t).
The diagonal mask is one fused [128,1024] bf16 multiply with a
precomputed mask tile.  All PSUM->SBUF copies run on DVE; the scalar
engine does nothing but exp.

v3: the two heads of a pair are row-tiled onto the top/bottom 64-row
halves of the PE array (DK=64 contraction), so a score-tile pair costs
one matmul slot instead of two zero-padded ones.
"""

import numpy as np
import ml_dtypes

B, S, D, H = 4, 2048, 1024, 16
DK = D // H
N_CORES = 8
HEADS_PER_CORE = H // 2


def build_attention_core(S=2048, DIN=1024, NH=8, DOUT=1024, aug_bias=False):
    import concourse.bacc as bacc
    import concourse.bass as bass
    import concourse.mybir as mybir
    import concourse.tile as tile

    fp32 = mybir.dt.float32
    bf16 = mybir.dt.bfloat16

    NP = NH // 2              # head pairs
    DC = NH * DK              # concat head dim on this core
    VW = 128                  # per-head V slot: [V(64) ones(1) pad(63)]
    NT = S // 128             # t tiles (key/value positions)
    NQ = S // 512             # q chunks of 512
    KA = DIN + 1 if aug_bias else DIN
    NK = (KA + 127) // 128    # contraction tiles for projections
    QT = S // 128             # output q tiles

    assert S % 512 == 0 and DIN % 128 == 0 and DOUT == 1024

    nc = bacc.Bacc(None, target_bir_lowering=False, debug=False)

    xq = nc.dram_tensor("xq", [KA, S], bf16, kind="ExternalInput")
    xk = nc.dram_tensor("xk", [KA, S], bf16, kind="ExternalInput")
    xv = nc.dram_tensor("xv", [KA, S], bf16, kind="ExternalInput")
    wq = nc.dram_tensor("wq", [KA, DC], bf16, kind="ExternalInput")
    wk = nc.dram_tensor("wk", [KA, DC], bf16, kind="ExternalInput")
    wv = nc.dram_tensor("wv", [KA, DC], bf16, kind="ExternalInput")
    wo = nc.dram_tensor("wo", [DC, DOUT], bf16, kind="ExternalInput")
    dmk = nc.dram_tensor("dmk", [128, 4 * 1024], bf16, kind="ExternalInput")
    outp = nc.dram_tensor("outp", [S, DOUT], fp32, kind="ExternalOutput")

    def ksz(k):  # rows in contraction tile k
        return min(128, KA - k * 128)

    scale = float(1.0 / np.sqrt(DK))

    with tile.TileContext(nc) as tc:
        with (
            tc.tile_pool(name="persist", bufs=1) as persist,
            tc.tile_pool(name="xin", bufs=32) as xin,
            tc.tile_pool(name="win", bufs=1) as win,
            tc.tile_pool(name="epool", bufs=6) as epool,
            tc.tile_pool(name="npool", bufs=2) as npool,
            tc.tile_pool(name="opool", bufs=2) as opool,
            tc.tile_pool(name="scps", bufs=2, space="PSUM") as scps,
            tc.tile_pool(name="otaps", bufs=2, space="PSUM") as otaps,
            tc.tile_pool(name="otbps", bufs=2, space="PSUM") as otbps,
        ):
            # ---- persistent SBUF tensors -------------------------------
            qht = persist.tile([128, NP * S], bf16, tag="qht")        # pair-major
            kht = persist.tile([128, NP * S], bf16, tag="kht")        # pair-major
            vh = persist.tile([128, NH * NT * VW], bf16, tag="vh")    # head-major
            ot = persist.tile([128, NP * S], bf16, tag="ot")
            dmask = persist.tile([128, 4 * 1024], bf16, tag="dmask")
            wo_sb = persist.tile([128, NP * DOUT], bf16, tag="wo")

            # K-path DMAs first so the first projection chain starts ASAP;
            # everything else is emitted behind them.
            wt = {}

            def dma_w(which, wdram):
                for k in range(NK):
                    wtile = win.tile([128, DC], bf16, tag=f"w{which}{k}")
                    nc.sync.dma_start(
                        wtile[: ksz(k), :], wdram[k * 128: k * 128 + ksz(k), :]
                    )
                    wt[(which, k)] = wtile

            vh4 = vh.rearrange("p (h t c) -> p h t c", t=NT, c=VW)

            # ---------------- helper emitters ---------------------------
            def dma_x(xdram, n, tagless_list):
                """DMA k-tiles of column-chunk n of xdram into pool tiles."""
                tiles = []
                for k in range(NK):
                    t_ = xin.tile([128, 512], bf16, tag="x")
                    nc.sync.dma_start(
                        t_[: ksz(k), :],
                        xdram[k * 128: k * 128 + ksz(k), n * 512:(n + 1) * 512],
                    )
                    tiles.append(t_)
                tagless_list.append(tiles)
                return tiles

            def proj_kq_fillers(which, p, n, xtiles):
                """Closures (one matmul each) for a K/Q projection chain."""
                pool = otbps if which == "q" else otaps
                tag = "otb" if which == "q" else "ota"
                box = {}

                def mk(k):
                    def emit():
                        if k == 0:
                            box["ps"] = pool.tile([128, 512], fp32, tag=tag, name="pjps")
                        ps = box["ps"]
                        nc.tensor.matmul(
                            ps[:],
                            wt[(which, k)][: ksz(k), p * 128:(p + 1) * 128],
                            xtiles[k][: ksz(k), :],
                            start=(k == 0),
                            stop=(k == NK - 1),
                        )
                        if k == NK - 1:
                            dst = qht if which == "q" else kht
                            nc.vector.tensor_copy(
                                dst[:, p * S + n * 512: p * S + (n + 1) * 512],
                                ps[:],
                            )

                    return emit

                return [mk(k) for k in range(NK)]

            def proj_kq(which, p, n, xtiles):
                for f in proj_kq_fillers(which, p, n, xtiles):
                    f()

            def proj_v_fillers(n, xtiles):
                """Closures (one t-tile each) for chunk n's V projection."""

                def mk(tt):
                    def emit():
                        t = n * 4 + tt
                        ps = scps.tile([128, 1024], fp32, tag="sc")
                        for k in range(NK):
                            nc.tensor.matmul(
                                ps[:, 0:512],
                                xtiles[k][: ksz(k), tt * 128:(tt + 1) * 128],
                                wt[("v", k)][: ksz(k), :],
                                start=(k == 0),
                                stop=(k == NK - 1),
                            )
                        nc.vector.tensor_copy(
                            vh4[:, :, t, 0:DK],
                            ps[:, 0:512].rearrange("p (h c) -> p h c", c=DK),
                        )

                    return emit

                return [mk(tt) for tt in range(4)]

            def proj_v(n, xtiles):
                """V projection for the 4 t-tiles of chunk n (all heads)."""
                for f in proj_v_fillers(n, xtiles):
                    f()

            def sc_mm(p, n):
                """Emit the score matmul pair for (pair p, chunk n, tile t).

                DK=64, so each head's score matmul only needs 64 contraction
                rows; the two heads of a pair are row-tiled onto halves of
                the PE array (tile_position (0,0) / (64,0), inferred from the
                base partitions) and run concurrently."""
                qof = p * S + n * 512
                kof = p * S

                def emit(t):
                    sc = scps.tile([128, 1024], fp32, tag="sc")
                    nc.tensor.matmul(
                        sc[:, 0:512],
                        kht[0:64, kof + t * 128: kof + (t + 1) * 128],
                        qht[0:64, qof: qof + 512],
                        start=True, stop=True,
                    )
                    nc.tensor.matmul(
                        sc[:, 512:1024],
                        kht[64:128, kof + t * 128: kof + (t + 1) * 128],
                        qht[64:128, qof: qof + 512],
                        start=True, stop=True,
                    )
                    return sc

                return emit

            pend = {}

            def attn_chunk(p, n, fillers=(), nxt=None):
                """Attention for pair p, q-chunk n (512 q positions).

                fillers: closures, each emitting ~1 PE matmul, interleaved one
                per t-iteration to fill the exp-wait slack.
                nxt: (p', n') of the following chunk; its first score matmul
                pair is emitted just before this chunk's last PV so the
                scalar engine never idles across the boundary."""
                qof = p * S + n * 512
                ota = otaps.tile([128, 512], fp32, tag="ota")
                otb = otbps.tile([128, 512], fp32, tag="otb")
                mine = sc_mm(p, n)
                sc_cur = pend.pop("sc", None)
                if sc_cur is None:
                    sc_cur = mine(0)
                fq = list(fillers)
                nf = len(fq)
                emitted = 0
                for t in range(NT):
                    e = epool.tile([128, 1024], bf16, tag="e")
                    nc.scalar.activation(
                        e[:], sc_cur[:], mybir.ActivationFunctionType.Exp,
                        scale=scale,
                    )
                    off = t * 128 - n * 512
                    if 0 <= off < 512:
                        d = off // 128
                        nc.vector.tensor_mul(
                            e[:], e[:], dmask[:, d * 1024:(d + 1) * 1024]
                        )
                    if t < NT - 1:
                        sc_cur = mine(t + 1)
                    elif nxt is not None:
                        pend["sc"] = sc_mm(*nxt)(0)
                    while emitted < ((t + 1) * nf) // NT:
                        fq[emitted]()
                        emitted += 1
                    va = ((2 * p) * NT + t) * VW
                    vb = ((2 * p + 1) * NT + t) * VW
                    nc.tensor.matmul(
                        ota[:], vh[:, va: va + VW], e[:, 0:512],
                        start=(t == 0), stop=(t == NT - 1),
                    )
                    nc.tensor.matmul(
                        otb[:], vh[:, vb: vb + VW], e[:, 512:1024],
                        start=(t == 0), stop=(t == NT - 1),
                    )
                while emitted < nf:
                    fq[emitted]()
                    emitted += 1

                # normalize (denominators on PSUM row 64)
                rd = npool.tile([128, 1024], fp32, tag="rd")
                nc.vector.reciprocal_approx_fast(rd[:, 0:512], ota[:])
                nc.vector.reciprocal_approx_fast(rd[:, 512:1024], otb[:])
                nc.sync.dma_start(rd[0:1, 0:512], rd[64:65, 0:512])
                nc.sync.dma_start(rd[0:1, 512:1024], rd[64:65, 512:1024])
                bca = npool.tile([64, 512], fp32, tag="bca")
                bcb = npool.tile([64, 512], fp32, tag="bcb")
                nc.gpsimd.partition_broadcast(bca[:], rd[0:1, 0:512], channels=64)
                nc.gpsimd.partition_broadcast(bcb[:], rd[0:1, 512:1024], channels=64)
                nc.vector.tensor_mul(ot[0:64, qof: qof + 512], ota[0:64, :], bca[:])
                tmpb = npool.tile([64, 512], bf16, tag="tmpb")
                nc.vector.tensor_mul(tmpb[:], otb[0:64, :], bcb[:])
                nc.sync.dma_start(ot[64:128, qof: qof + 512], tmpb[:])

            def out_qt_closure(qt):
                """One output-projection q-subtile (8 matmuls + copy + DMA)."""

                def emit():
                    ps = scps.tile([128, 1024], fp32, tag="sc", name="cps")
                    for nd in range(2):
                        for p in range(NP):
                            nc.tensor.matmul(
                                ps[:, nd * 512:(nd + 1) * 512],
                                ot[:, p * S + qt * 128: p * S + (qt + 1) * 128],
                                wo_sb[:, p * DOUT + nd * 512: p * DOUT + nd * 512 + 512],
                                start=(p == 0), stop=(p == NP - 1),
                            )
                    osb = opool.tile([128, 1024], fp32, tag="osb", name="osb")
                    nc.vector.tensor_copy(osb[:], ps[:])
                    nc.sync.dma_start(outp[qt * 128:(qt + 1) * 128, :], osb[:])

                return emit

            def out_chunk(n):
                for qt in range(n * 4, n * 4 + 4):
                    out_qt_closure(qt)()

            # ---------------- emission ----------------------------------
            keep = []
            # head: minimal critical path to the first exp —
            # wk+xk -> K(p0, all chunks); wq+xq0 -> Q(p0, c0); first scores.
            dma_w("k", wk)
            kx = [dma_x(xk, n, keep) for n in range(NQ)]
            dma_w("q", wq)
            qx0 = dma_x(xq, 0, keep)
            for n in range(NQ):
                proj_kq("k", 0, n, kx[n])
            proj_kq("q", 0, 0, qx0)
            # everything else lands behind the K/Q critical path
            dma_w("v", wv)
            vx = [dma_x(xv, n, keep) for n in range(NQ)]
            nc.vector.memset(vh4[:, :, :, 64:65], 1.0)  # ones columns only
            nc.sync.dma_start(dmask[:], dmk[:])
            for p in range(NP):
                nc.sync.dma_start(
                    wo_sb[:, p * DOUT:(p + 1) * DOUT], wo[p * 128:(p + 1) * 128, :]
                )
            qx = [None] + [dma_x(xq, n, keep) for n in range(1, NQ)]
            # rolling x prefetch for the first stage's fillers
            xbuf = {0: (dma_x(xk, 0, keep), dma_x(xq, 0, keep))}
            # V: chunks 0,1 before the t-loop; chunks 2,3 as stage-(0,0)
            # fillers (PV consumes V tile t at iteration t, so the later
            # tiles can land mid-chunk).
            proj_v(0, vx[0])
            proj_v(1, vx[1])
            vfill = proj_v_fillers(2, vx[2]) + proj_v_fillers(3, vx[3])
            qfill = {n: proj_kq_fillers("q", 0, n, qx[n]) for n in (1, 2, 3)}

            # stages: chunk (p, n) with next-chunk score lookahead,
            # next-pair projections (or output-projection subtiles in the
            # last stage) interleaved into the t-loop, and x chunks
            # DMA'd one chunk ahead
            order = [(p, n) for p in range(NP) for n in range(NQ)]
            for idx, (p, n) in enumerate(order):
                if idx + 1 < len(order) and order[idx + 1][0] < NP - 1:
                    n2 = order[idx + 1][1]
                    xbuf[idx + 1] = (dma_x(xk, n2, keep), dma_x(xq, n2, keep))
                fillers = []
                if p == 0 and n == 0:
                    fillers = list(vfill) + qfill[1]
                elif p == 0 and n in (1, 2):
                    fillers = list(qfill[n + 1])
                if p < NP - 1:
                    kxn, qxn = xbuf.pop(idx)
                    fillers = (
                        fillers
                        + proj_kq_fillers("k", p + 1, n, kxn)
                        + proj_kq_fillers("q", p + 1, n, qxn)
                    )
                elif n > 0:
                    fillers = [out_qt_closure(qt)
                               for qt in range((n - 1) * 4, n * 4)]
                nxt = order[idx + 1] if idx + 1 < len(order) else None
                attn_chunk(p, n, fillers, nxt)
            out_chunk(NQ - 1)

    nc.compile()
    return nc


def _bf16(a):
    return np.ascontiguousarray(a).astype(ml_dtypes.bfloat16)


def _build_dmask():
    m = np.ones((128, 4 * 1024), np.float32)
    for d in range(4):
        for i in range(128):
            m[i, d * 1024 + d * 128 + i] = 0.0
            m[i, d * 1024 + 512 + d * 128 + i] = 0.0
    return _bf16(m)


def _prep_core_inputs(q, k, v, Wq, bq, Wk, bk, Wv, bv, Wo, aug_bias):
    """Per-core host-side slicing/transposition. Returns list of 8 dicts."""
    dmk = _build_dmask()
    maps = []
    for c in range(N_CORES):
        b = c // 2
        h0 = (c % 2) * HEADS_PER_CORE
        r0, r1 = h0 * DK, (h0 + HEADS_PER_CORE) * DK
        m = {}
        for name, x in (("xq", q[b]), ("xk", k[b]), ("xv", v[b])):
            xt = x.T  # [D, S]
            if aug_bias:
                xt = np.concatenate([xt, np.ones((1, S), np.float32)], axis=0)
            m[name] = _bf16(xt)
        for name, W, bias in (("wq", Wq, bq), ("wk", Wk, bk), ("wv", Wv, bv)):
            wtm = W[r0:r1, :].T  # [D, DC]
            if aug_bias:
                wtm = np.concatenate([wtm, bias[None, r0:r1]], axis=0)
            m[name] = _bf16(wtm)
        m["wo"] = _bf16(Wo[:, r0:r1].T)  # [DC, D]
        m["dmk"] = dmk
        maps.append(m)
    return maps


_PROGRAM_CACHE = {}


def _get_program(aug_bias):
    if aug_bias not in _PROGRAM_CACHE:
        _PROGRAM_CACHE[aug_bias] = build_attention_core(
            S=S, DIN=D, NH=HEADS_PER_CORE, DOUT=D, aug_bias=aug_bias
        )
    return _PROGRAM_CACHE[aug_bias]


def _reference_fallback(q, k, v, Wq, bq, Wk, bk, Wv, bv, Wo, bo, mask):
    """Pure-numpy fallback for unexpected mask patterns."""
    out = np.empty((B, S, D), np.float32)
    msk = np.broadcast_to(mask.reshape(mask.shape[-2], mask.shape[-1]), (S, S))
    for b in range(B):
        qh = (q[b] @ Wq.T + bq).reshape(S, H, DK).transpose(1, 0, 2)
        kh = (k[b] @ Wk.T + bk).reshape(S, H, DK).transpose(1, 0, 2)
        vh = (v[b] @ Wv.T + bv).reshape(S, H, DK).transpose(1, 0, 2)
        acc = np.empty((H, S, DK), np.float32)
        for h in range(H):
            s = (qh[h] @ kh[h].T) / np.float32(np.sqrt(DK))
            s = np.where(msk == 0, np.finfo(np.float32).min, s)
            s = s - s.max(axis=-1, keepdims=True)
            e = np.exp(s)
            p = e / e.sum(axis=-1, keepdims=True)
            acc[h] = p @ vh[h]
        o = acc.transpose(1, 0, 2).reshape(S, D)
        out[b] = o @ Wo.T + bo
    return out


def kernel(q, k, v, Wq, bq, Wk, bk, Wv, bv, Wo, bo, mask, _trace=False):
    from concourse.bass_utils import run_bass_kernel_spmd

    q = np.asarray(q, np.float32)
    k = np.asarray(k, np.float32)
    v = np.asarray(v, np.float32)
    Wq, bq = np.asarray(Wq, np.float32), np.asarray(bq, np.float32)
    Wk, bk = np.asarray(Wk, np.float32), np.asarray(bk, np.float32)
    Wv, bv = np.asarray(Wv, np.float32), np.asarray(bv, np.float32)
    Wo, bo = np.asarray(Wo, np.float32), np.asarray(bo, np.float32)
    mask = np.asarray(mask)

    expected_mask = 1 - np.eye(S, dtype=np.int32)
    if not np.array_equal(mask.reshape(-1, S, S)[0].astype(np.int32), expected_mask):
        return _reference_fallback(q, k, v, Wq, bq, Wk, bk, Wv, bv, Wo, bo, mask)

    aug_bias = bool(np.any(bq) or np.any(bk) or np.any(bv))
    nc = _get_program(aug_bias)
    in_maps = _prep_core_inputs(q, k, v, Wq, bq, Wk, bk, Wv, bv, Wo, aug_bias)
    res = run_bass_kernel_spmd(
        nc, in_maps, core_ids=list(range(N_CORES)), trace=_trace
    )
    out = np.empty((B, S, D), np.float32)
    for b in range(B):
        out[b] = res.results[2 * b]["outp"] + res.results[2 * b + 1]["outp"] + bo
    if _trace:
        kernel.last_results = res
    return out

